# revision 11
# baseline (speedup 1.0000x reference)
"""CARAFE upsampling (k=5, x2, C=256) as a Bass/Tile kernel on 8 NeuronCores.

Math (per output pixel):
  out[b, Y, X, c] = sum_{ky,kx} softmax(masks[b,Y,X,:])[ky*5+kx]
                    * feat[b, Y//2+ky-2, X//2+kx-2, c]       (zero padded)

Mapping: pure data parallel over (batch, 32-output-row strips) -> 8 cores,
no collectives.

v4 design:
* Softmax fully on host (fp32): device does only matmul + PSUM->SBUF copy
  + DMA.  No exp, no reciprocal, no ones-column.
* Each core's 32 output rows split into FOUR interleaved strips of 8 rows.
  Step order round-robins strips (A0 B0 C0 D0 A1 ...), so a strip's window
  ring slot is reused only 4 steps later -> feature prefetch has ~3 steps
  of latency slack and every feature row is loaded exactly once per rep.
* Per step (strip q, pair j): 8 matmuls M=32,K=100,N=256 col-tiled over 4
  PE column groups; both output rows accumulate into one PSUM bank
  [128, 512]; a single PSUM->SBUF bf16 copy (alternating DVE/ACT) evicts.
* Masks arrive as a host-built banded lhsT [100, 4096] of softmax weights
  over a zero background (zeros contribute nothing to the dot product).
  lhsT column = ((2u+v)*8 + b*2 + r)*16 + (q*4+j); partition
  p = ((j+ky)%5)*20 + (u+kx).  Each matmul's 32 weight columns sit at
  constant stride 128 (walrus requires a single free dim on weights).
* The banded matrix is only 25% occupied, so the device ships just the
  real entries: the lhsT tiles are zeroed ONCE in the prologue, and per
  rep 20 small DMAs overlay the per-t compact blocks (for partition
  p = s*20 + t the real entries are the contiguous columns
  [umin(t)*256, (umin(t)+nu(t))*256) with umin = max(0,t-4),
  nu = min(15,t)-umin+1).  Mask HBM traffic: 819KB -> 205KB per rep.
* Output x-major SBUF tiles [x=128, 4KiB] stored with 2 big DMAs per rep.
"""

import sys

for _p in ("/opt/trn_rl_repo",):
    if _p not in sys.path:
        sys.path.insert(0, _p)

import numpy as np

B = 2
H_IN = 64
W_IN = 64
C = 256
H_OUT = 128
W_OUT = 128
KK = 25
N_CORES = 8
ROWS_PER_CORE = H_OUT * B // N_CORES  # 32 output rows
NSTRIP = 4
PAIRS_PER_STRIP = 4
SLAB = 8          # feature rows a strip touches (4 + 2 halo each side)
NBLK = 4          # X blocks per row
TW = 20           # t window width per block
KDIM = 5 * TW     # matmul contraction size (5 ring slots x 20 cols)
NSTEP = NSTRIP * PAIRS_PER_STRIP  # 16


def _t_blocks():
    """Per-t compact mask blocks: (t, umin, nu, flat_offset) + total size."""
    blocks = []
    off = 0
    for t in range(TW):
        umin = max(0, t - 4)
        nu = min(15, t) - umin + 1
        blocks.append((t, umin, nu, off))
        off += 5 * nu * C
    return blocks, off


T_BLOCKS, MSK_TOTAL = _t_blocks()

_NC_CACHE = {}


def _build_nc(reps=1):
    import concourse.bacc as bacc
    import concourse.mybir as mybir
    from concourse import tile

    dt = mybir.dt
    f32 = dt.float32
    bf16 = dt.bfloat16

    nc = bacc.Bacc("TRN2", target_bir_lowering=False, debug=False,
                   num_devices=N_CORES)
    feat = nc.dram_tensor("feat", [NSTRIP, SLAB, TW, NBLK, C], bf16,
                          kind="ExternalInput")
    mlhs = nc.dram_tensor("mlhs", [MSK_TOTAL], bf16,
                          kind="ExternalInput")
    # out[x, q, g, (j_g r c)]: Y_local = q*8 + 4g + 2j_g + r
    out = nc.dram_tensor("out", [W_OUT, NSTRIP, 2, 4 * C], bf16,
                         kind="ExternalOutput")

    AP = type(feat[:])

    with tile.TileContext(nc) as tc:
        with (
            tc.tile_pool(name="big", bufs=1) as big,
            tc.tile_pool(name="psum", bufs=4, space="PSUM") as psumpool,
        ):
            lws = [big.tile([KDIM, NSTEP * C], bf16, tag=f"lw{i}",
                            name=f"lw{i}") for i in range(2)]
            wnds = [big.tile([KDIM, NBLK * C], bf16, tag=f"wnd{q}",
                             name=f"wnd{q}") for q in range(NSTRIP)]
            ots = [big.tile([W_OUT, ROWS_PER_CORE * C // 2], bf16,
                            tag=f"ot{g}", name=f"ot{g}") for g in range(2)]

            # zero the banded-lhsT background once (off the rep loop)
            nc.vector.memset(lws[0][:, :], 0.0)
            nc.gpsimd.memset(lws[1][:, :], 0.0)

            for rep in range(reps):
                lw = lws[rep % 2]
                lwf = lw[:]

                # masks: overlay only the real entries onto the zeroed
                # background -- 20 small HWDGE DMAs (~205KB total)
                for t, umin, nu, off in T_BLOCKS:
                    dst = AP(tensor=lwf.tensor,
                             offset=lwf.offset + t * (NSTEP * C)
                             + umin * 256,
                             ap=[[TW * NSTEP * C, 5], [1, nu * 256]])
                    eng = nc.scalar if t % 2 == 0 else nc.sync
                    eng.dma_start(
                        out=dst,
                        in_=mlhs[off:off + 5 * nu * 256].rearrange(
                            "(s e) -> s e", s=5))

                # window prologues: rows 0-4 of each strip -> slots 0-4
                peng = [nc.sync, nc.sync, nc.gpsimd, nc.gpsimd]
                for q in range(NSTRIP):
                    peng[q].dma_start(
                        out=wnds[q][:, :],
                        in_=feat[q, 0:5].rearrange("r t b c -> (r t) (b c)"))

                for j in range(PAIRS_PER_STRIP):
                    g, jg = divmod(j, 2)
                    ot = ots[g]
                    for q in range(NSTRIP):
                        s_idx = j * NSTRIP + q
                        wnd = wnds[q]
                        ps = psumpool.tile([128, 2 * C], f32, tag="ps",
                                           name="ps")
                        for r in range(2):
                            for b in range(NBLK):
                                lhsT = AP(
                                    tensor=lwf.tensor,
                                    offset=lwf.offset + (b * 2 + r) * 16
                                    + (q * PAIRS_PER_STRIP + j),
                                    ap=[[NSTEP * C, KDIM], [128, 32]])
                                nc.tensor.matmul(
                                    ps[32 * b:32 * (b + 1),
                                       r * C:(r + 1) * C],
                                    lhsT, wnd[:, b * C:(b + 1) * C],
                                    start=True, stop=True,
                                    tile_position=(0, 32 * b))
                        # single eviction per step, alternating engines
                        osl = ot[:, q * 1024 + jg * 512:
                                 q * 1024 + jg * 512 + 512]
                        if s_idx % 2 == 0:
                            nc.vector.tensor_copy(out=osl, in_=ps[:, :])
                        else:
                            nc.scalar.copy(out=osl, in_=ps[:, :])
                        # prefetch this strip's next ring row (j+5 -> slot j)
                        if j < 3:
                            row = j + 5
                            slot = row % 5
                            peng[q].dma_start(
                                out=wnd[slot * TW:(slot + 1) * TW, :],
                                in_=feat[q, row].rearrange(
                                    "t b c -> t (b c)"))
                    if j % 2 == 1:
                        # store the group's 16 output rows in one DMA
                        nc.gpsimd.dma_start(
                            out=out[:, :, g, :],
                            in_=ots[g][:].rearrange("x (q e) -> x q e",
                                                    q=NSTRIP))

    nc.compile()
    return nc


def get_nc(reps=1):
    key = reps
    if key not in _NC_CACHE:
        _NC_CACHE[key] = _build_nc(reps)
    return _NC_CACHE[key]


_IDX_CACHE = {}


def _mlhs_idx():
    """Flat scatter/gather indices mapping softmax weights -> banded lhsT."""
    if "mlhs" not in _IDX_CACHE:
        shp = (NSTRIP, PAIRS_PER_STRIP, 5, 5, 16, NBLK, 2, 2)
        q, j, ky, kx, u, b, v, r = np.indices(shp)
        p = ((j + ky) % 5) * TW + u + kx
        col = (((2 * u + v) * 8 + b * 2 + r) * 16
               + (q * PAIRS_PER_STRIP + j))
        row = q * 8 + 2 * j + r
        colx = 32 * b + 2 * u + v
        tap = 5 * ky + kx
        scatter = (p * (NSTEP * C) + col).ravel()
        gather = (row * (W_OUT * KK) + colx * KK + tap).ravel()
        _IDX_CACHE["mlhs"] = (scatter, gather)
    return _IDX_CACHE["mlhs"]


def shard_inputs(features, masks):
    """Full inputs -> per-core input maps.

    Host prep: softmax over mask taps (fp32), scatter into the banded
    zero-background lhsT; features window-duplicated into per-strip slabs.
    """
    import ml_dtypes

    bf16 = ml_dtypes.bfloat16
    features = np.asarray(features, np.float32)
    masks = np.asarray(masks, np.float32)

    # softmax over the 25 taps, once for the full tensor
    m = masks - masks.max(axis=-1, keepdims=True)
    np.exp(m, out=m)
    m /= m.sum(axis=-1, keepdims=True)

    # zero-padded features [B, H+4, W+4, C]
    padf = np.zeros((B, H_IN + 4, W_IN + 4, C), np.float32)
    padf[:, 2:2 + H_IN, 2:2 + W_IN] = features

    scatter, gather = _mlhs_idx()
    t = np.arange(TW).reshape(TW, 1)
    bb = np.arange(NBLK).reshape(1, NBLK)
    colidx = 16 * bb + t  # [TW, NBLK] padded-col index

    in_maps = []
    for core in range(N_CORES):
        bi, qc = divmod(core, 4)
        sy0 = 16 * qc  # first source row (unpadded) of this core

        # feature strips: fw[q, R, t, b, c] = padf[sy0 + 4q + R, 16b + t]
        fw = np.empty((NSTRIP, SLAB, TW, NBLK, C), np.float32)
        for q in range(NSTRIP):
            rows = padf[bi, sy0 + 4 * q: sy0 + 4 * q + SLAB]  # [8, W+4, C]
            fw[q] = rows[:, colidx, :]

        wrows = m[bi, ROWS_PER_CORE * qc: ROWS_PER_CORE * (qc + 1)]
        mlhs = np.zeros(KDIM * NSTEP * C, np.float32)
        mlhs[scatter] = wrows.ravel()[gather]
        full = mlhs.reshape(KDIM, NSTEP * C)
        msk = np.concatenate([
            full[t::TW, umin * 256:(umin + nu) * 256].ravel()
            for t, umin, nu, off in T_BLOCKS])

        in_maps.append({
            "feat": np.ascontiguousarray(fw.astype(bf16)),
            "mlhs": np.ascontiguousarray(msk.astype(bf16)),
        })
    return in_maps


def unshard_outputs(results):
    out = np.empty((B, H_OUT, W_OUT, C), np.float32)
    for core in range(N_CORES):
        bi, qc = divmod(core, 4)
        r = results[core]["out"].astype(np.float32)
        # [x, q, g, (j_g r c)] -> [q, g, j_g, r, x, c] -> [32, 128, C]
        r = r.reshape(W_OUT, NSTRIP, 2, 2, 2, C)
        r = r.transpose(1, 2, 3, 4, 0, 5).reshape(ROWS_PER_CORE, W_OUT, C)
        out[bi, ROWS_PER_CORE * qc: ROWS_PER_CORE * (qc + 1)] = r
    return out


def kernel(features, masks):
    from concourse.bass_utils import run_bass_kernel_spmd

    nc = get_nc()
    in_maps = shard_inputs(features, masks)
    res = run_bass_kernel_spmd(nc, in_maps, list(range(N_CORES)))
    return unshard_outputs(res.results)


# revision 22
# speedup vs baseline: 1.4027x; 1.4027x over previous
"""CARAFE upsampling (k=5, x2, C=256) as a Bass/Tile kernel on 8 NeuronCores.

Math (per output pixel):
  out[b, Y, X, c] = sum_{ky,kx} softmax(masks[b,Y,X,:])[ky*5+kx]
                    * feat[b, Y//2+ky-2, X//2+kx-2, c]       (zero padded)

Mapping: pure data parallel over (batch, 32-output-row strips) -> 8 cores,
no collectives.

v4 design:
* Softmax fully on host (fp32): device does only matmul + PSUM->SBUF copy
  + DMA.  No exp, no reciprocal, no ones-column.
* Each core's 32 output rows split into FOUR interleaved strips of 8 rows.
  Step order round-robins strips (A0 B0 C0 D0 A1 ...), so a strip's window
  ring slot is reused only 4 steps later -> feature prefetch has ~3 steps
  of latency slack and every feature row is loaded exactly once per rep.
* Per step (strip q, pair j): 8 matmuls M=32,K=100,N=256 col-tiled over 4
  PE column groups; both output rows accumulate into one PSUM bank
  [128, 512]; a single PSUM->SBUF bf16 copy (alternating DVE/ACT) evicts.
* Masks arrive as a host-built banded lhsT [100, 4096] of softmax weights
  over a zero background (zeros contribute nothing to the dot product).
  lhsT column = ((2u+v)*8 + b*2 + r)*16 + (q*4+j); partition
  p = ((j+ky)%5)*20 + (u+kx).  Each matmul's 32 weight columns sit at
  constant stride 128 (walrus requires a single free dim on weights).
* The banded matrix is only 25% occupied, so the device ships just the
  real entries: the lhsT tiles are zeroed ONCE in the prologue, and per
  rep 20 small DMAs overlay the per-t compact blocks (for partition
  p the real entries are the contiguous columns [umin(t)*256,
  (umin(t)+nu(t))*256) with umin = max(0,t-4), nu = min(15,t)-umin+1).
  Mask HBM traffic: 819KB -> 205KB per rep.
* Contraction partitions are T-MAJOR: p = t*5 + s (s = ring slot).
  This makes each overlay DMA's partition range [5t, 5t+5) contiguous,
  so the Tile dep tracker (which collapses each access to one flat
  [start, end] interval) sees the 20 overlays as disjoint and lets them
  run in parallel -- with s-major layout they chained serially on
  false WAW edges (~700ns/link, measured).
* Output x-major SBUF tiles [x=128, 4KiB] stored with 2 big DMAs per rep.
"""

import sys

for _p in ("/opt/trn_rl_repo",):
    if _p not in sys.path:
        sys.path.insert(0, _p)

import numpy as np

B = 2
H_IN = 64
W_IN = 64
C = 256
H_OUT = 128
W_OUT = 128
KK = 25
N_CORES = 8
ROWS_PER_CORE = H_OUT * B // N_CORES  # 32 output rows
NSTRIP = 4
PAIRS_PER_STRIP = 4
SLAB = 8          # feature rows a strip touches (4 + 2 halo each side)
NBLK = 4          # X blocks per row
TW = 20           # t window width per block
KDIM = 5 * TW     # matmul contraction size (5 ring slots x 20 cols)
NSTEP = NSTRIP * PAIRS_PER_STRIP  # 16


def _t_blocks():
    """Per-t compact mask blocks: (t, umin, nu, flat_offset) + total size."""
    blocks = []
    off = 0
    for t in range(TW):
        umin = max(0, t - 4)
        nu = min(15, t) - umin + 1
        blocks.append((t, umin, nu, off))
        off += 5 * nu * C
    return blocks, off


T_BLOCKS, MSK_TOTAL = _t_blocks()

_NC_CACHE = {}


def _build_nc(reps=1):
    import concourse.bacc as bacc
    import concourse.mybir as mybir
    from concourse import tile

    dt = mybir.dt
    f32 = dt.float32
    bf16 = dt.bfloat16

    nc = bacc.Bacc("TRN2", target_bir_lowering=False, debug=False,
                   num_devices=N_CORES)
    feat = nc.dram_tensor("feat", [NSTRIP, TW, SLAB, NBLK, C], bf16,
                          kind="ExternalInput")
    mlhs = nc.dram_tensor("mlhs", [MSK_TOTAL], bf16,
                          kind="ExternalInput")
    # out[x, q, g, (j_g r c)]: Y_local = q*8 + 4g + 2j_g + r
    out = nc.dram_tensor("out", [W_OUT, NSTRIP, 2, 4 * C], bf16,
                         kind="ExternalOutput")

    AP = type(feat[:])

    with tile.TileContext(nc) as tc:
        with (
            tc.tile_pool(name="big", bufs=1) as big,
            tc.tile_pool(name="psum", bufs=4, space="PSUM") as psumpool,
        ):
            lws = [big.tile([KDIM, NSTEP * C], bf16, tag=f"lw{i}",
                            name=f"lw{i}") for i in range(2)]
            wnds = [big.tile([KDIM, NBLK * C], bf16, tag=f"wnd{q}",
                             name=f"wnd{q}") for q in range(NSTRIP)]
            ots = [big.tile([W_OUT, ROWS_PER_CORE * C // 2], bf16,
                            tag=f"ot{g}", name=f"ot{g}") for g in range(2)]

            # zero the banded-lhsT background once (off the rep loop)
            nc.vector.memset(lws[0][:, :], 0.0)
            nc.gpsimd.memset(lws[1][:, :], 0.0)

            for rep in range(reps):
                lw = lws[rep % 2]
                lwf = lw[:]

                # masks: overlay only the real entries onto the zeroed
                # background -- 20 small HWDGE DMAs (~205KB total);
                # t-major partitions make the 20 writes disjoint
                for t, umin, nu, off in T_BLOCKS:
                    dst = AP(tensor=lwf.tensor,
                             offset=lwf.offset + 5 * t * (NSTEP * C)
                             + umin * 256,
                             ap=[[NSTEP * C, 5], [1, nu * 256]])
                    eng = nc.scalar if t % 2 == 0 else nc.sync
                    eng.dma_start(
                        out=dst,
                        in_=mlhs[off:off + 5 * nu * 256].rearrange(
                            "(s e) -> s e", s=5))

                # window prologues: rows 0-4 of each strip -> slots 0-4
                # (t-major partitions: transpose (r, t) on the DRAM side)
                peng = [nc.sync, nc.sync, nc.gpsimd, nc.gpsimd]
                for q in range(NSTRIP):
                    peng[q].dma_start(
                        out=wnds[q][:, :],
                        in_=feat[q, :, 0:5].rearrange(
                            "t r b c -> t r (b c)"))

                for j in range(PAIRS_PER_STRIP):
                    g, jg = divmod(j, 2)
                    ot = ots[g]
                    for q in range(NSTRIP):
                        s_idx = j * NSTRIP + q
                        wnd = wnds[q]
                        ps = psumpool.tile([128, 2 * C], f32, tag="ps",
                                           name="ps")
                        for r in range(2):
                            for b in range(NBLK):
                                lhsT = AP(
                                    tensor=lwf.tensor,
                                    offset=lwf.offset + (b * 2 + r) * 16
                                    + (q * PAIRS_PER_STRIP + j),
                                    ap=[[NSTEP * C, KDIM], [128, 32]])
                                nc.tensor.matmul(
                                    ps[32 * b:32 * (b + 1),
                                       r * C:(r + 1) * C],
                                    lhsT, wnd[:, b * C:(b + 1) * C],
                                    start=True, stop=True,
                                    tile_position=(0, 32 * b))
                        # single eviction per step, alternating engines
                        osl = ot[:, q * 1024 + jg * 512:
                                 q * 1024 + jg * 512 + 512]
                        if s_idx % 2 == 0:
                            nc.vector.tensor_copy(out=osl, in_=ps[:, :])
                        else:
                            nc.scalar.copy(out=osl, in_=ps[:, :])
                        # prefetch this strip's next ring row (j+5 -> slot j)
                        if j < 3:
                            row = j + 5
                            slot = row % 5
                            wap = wnd[:]
                            dst = AP(tensor=wap.tensor,
                                     offset=wap.offset + slot * (NBLK * C),
                                     ap=[[5 * NBLK * C, TW], [1, NBLK * C]])
                            peng[q].dma_start(
                                out=dst,
                                in_=feat[q, :, row].rearrange(
                                    "t b c -> t (b c)"))
                    if j % 2 == 1:
                        # store the group's 16 output rows in one DMA
                        nc.gpsimd.dma_start(
                            out=out[:, :, g, :],
                            in_=ots[g][:].rearrange("x (q e) -> x q e",
                                                    q=NSTRIP))

    nc.compile()
    return nc


def get_nc(reps=1):
    key = reps
    if key not in _NC_CACHE:
        _NC_CACHE[key] = _build_nc(reps)
    return _NC_CACHE[key]


_IDX_CACHE = {}


def _mlhs_idx():
    """Flat scatter/gather indices mapping softmax weights -> banded lhsT."""
    if "mlhs" not in _IDX_CACHE:
        shp = (NSTRIP, PAIRS_PER_STRIP, 5, 5, 16, NBLK, 2, 2)
        q, j, ky, kx, u, b, v, r = np.indices(shp)
        p = (u + kx) * 5 + (j + ky) % 5  # t-major partitions
        col = (((2 * u + v) * 8 + b * 2 + r) * 16
               + (q * PAIRS_PER_STRIP + j))
        row = q * 8 + 2 * j + r
        colx = 32 * b + 2 * u + v
        tap = 5 * ky + kx
        scatter = (p * (NSTEP * C) + col).ravel()
        gather = (row * (W_OUT * KK) + colx * KK + tap).ravel()
        _IDX_CACHE["mlhs"] = (scatter, gather)
    return _IDX_CACHE["mlhs"]


def shard_inputs(features, masks):
    """Full inputs -> per-core input maps.

    Host prep: softmax over mask taps (fp32), scatter into the banded
    zero-background lhsT; features window-duplicated into per-strip slabs.
    """
    import ml_dtypes

    bf16 = ml_dtypes.bfloat16
    features = np.asarray(features, np.float32)
    masks = np.asarray(masks, np.float32)

    # softmax over the 25 taps, once for the full tensor
    m = masks - masks.max(axis=-1, keepdims=True)
    np.exp(m, out=m)
    m /= m.sum(axis=-1, keepdims=True)

    # zero-padded features [B, H+4, W+4, C]
    padf = np.zeros((B, H_IN + 4, W_IN + 4, C), np.float32)
    padf[:, 2:2 + H_IN, 2:2 + W_IN] = features

    scatter, gather = _mlhs_idx()
    t = np.arange(TW).reshape(TW, 1)
    bb = np.arange(NBLK).reshape(1, NBLK)
    colidx = 16 * bb + t  # [TW, NBLK] padded-col index

    in_maps = []
    for core in range(N_CORES):
        bi, qc = divmod(core, 4)
        sy0 = 16 * qc  # first source row (unpadded) of this core

        # feature strips: fw[q, t, R, b, c] = padf[sy0 + 4q + R, 16b + t]
        fw = np.empty((NSTRIP, TW, SLAB, NBLK, C), np.float32)
        for q in range(NSTRIP):
            rows = padf[bi, sy0 + 4 * q: sy0 + 4 * q + SLAB]  # [8, W+4, C]
            fw[q] = rows[:, colidx, :].transpose(1, 0, 2, 3)

        wrows = m[bi, ROWS_PER_CORE * qc: ROWS_PER_CORE * (qc + 1)]
        mlhs = np.zeros(KDIM * NSTEP * C, np.float32)
        mlhs[scatter] = wrows.ravel()[gather]
        full = mlhs.reshape(KDIM, NSTEP * C)
        msk = np.concatenate([
            full[5 * t:5 * t + 5, umin * 256:(umin + nu) * 256].ravel()
            for t, umin, nu, off in T_BLOCKS])

        in_maps.append({
            "feat": np.ascontiguousarray(fw.astype(bf16)),
            "mlhs": np.ascontiguousarray(msk.astype(bf16)),
        })
    return in_maps


def unshard_outputs(results):
    out = np.empty((B, H_OUT, W_OUT, C), np.float32)
    for core in range(N_CORES):
        bi, qc = divmod(core, 4)
        r = results[core]["out"].astype(np.float32)
        # [x, q, g, (j_g r c)] -> [q, g, j_g, r, x, c] -> [32, 128, C]
        r = r.reshape(W_OUT, NSTRIP, 2, 2, 2, C)
        r = r.transpose(1, 2, 3, 4, 0, 5).reshape(ROWS_PER_CORE, W_OUT, C)
        out[bi, ROWS_PER_CORE * qc: ROWS_PER_CORE * (qc + 1)] = r
    return out


def kernel(features, masks):
    from concourse.bass_utils import run_bass_kernel_spmd

    nc = get_nc()
    in_maps = shard_inputs(features, masks)
    res = run_bass_kernel_spmd(nc, in_maps, list(range(N_CORES)))
    return unshard_outputs(res.results)
